# revision 58
# baseline (speedup 1.0000x reference)
"""Dense optical flow kernel for Trainium2, 8-core SPMD.

Pipeline (per core = one (sample, x-half) pair), x-polyphase layout
(x = 4j + p) so every correlation window read is a dense stride-1 run:

  frames -> gray/sobel features (row-polyphase ry, col-polyphase p)
  -> l2-normalize f2 (ACT abs_reciprocal_sqrt + DVE Newton)
  -> replicated window tensor f2px -> 15x15 windowed correlation (f32)
    on DVE with dense mults/adds and dense pairwise max trees
  -> first-argmax -> displacement grid -> separable gaussian smoothing
    (phase H-pass on DVE, banded-matmul V-pass on PE)
  -> direction normalize (ACT-heavy, no Newton) -> full-res flow.

Cross-partition row shifts for the vertical sobel go through PE
shift-matmuls (off-diagonal identity) instead of SBUF->SBUF DMA.
"""

import numpy as np

import concourse.bacc as bacc
import concourse.tile as tile
from concourse import mybir
from concourse.ap import AP
from concourse.bass_utils import run_bass_kernel_spmd

F32 = mybir.dt.float32
Alu = mybir.AluOpType
Act = mybir.ActivationFunctionType
AX = mybir.AxisListType

H = 512
W = 512
B = 4
XL = 288          # per-core padded column span
JL = 72           # XL / 4 (x-polyphase)
GXL = 68          # local anchor columns (64 + 2 halo each side)
NEG = np.float32(-1.0e30)
POS = np.float32(3.0e38)

# consts layout offsets (fp32 elements per partition)
OFF_XMASK = 0                   # [4ry, 4p, 72j] = 1152
OFF_GMASK = 1152                # [68]
OFF_BANDS = 1220                # [4t, 128y] = 512
OFF_WS = 1732                   # [15, 68] = 1020 (natural dx order)
OFF_SHDN = 2752                 # [128]
OFF_SHUP = 2880                 # [128]
OFF_WSY = 3008                  # [15, 68] = 1020 (compact-rowmax dy order)
NCONST = 4028

# rowmax compact slot order -> dy index (0..14 ~ dy=-7..7):
# slots 0:7 = f2px slots 1..7 (dy 0..6), 7:11 = f2px 12..15 (dy 11..14),
# 11:15 = ov0 early slots (dy 7..10)
DYIDX = [0, 1, 2, 3, 4, 5, 6, 11, 12, 13, 14, 7, 8, 9, 10]


# ----------------------------------------------------------------------------
# constants (host side)
# ----------------------------------------------------------------------------

def _gaussian_sep():
    ax = np.arange(15) - 7
    g = np.exp(-(ax.astype(np.float64) ** 2) / (2.0 * 2.5 ** 2))
    return (g / g.sum())


def _phase_weights():
    g = _gaussian_sep()
    Wp = np.zeros((4, 5), np.float64)
    for p in range(4):
        for t in range(15):
            Wp[p, (p + t - 7) // 4 + 2] += g[t]
    return Wp.astype(np.float32)


def _band_matrices():
    # bands[t][v, y]: out_row(128t+y) = sum_v band[v, y] * hp[v]
    Wp = _phase_weights()
    bands = np.zeros((4, 128, 128), np.float32)
    for t in range(4):
        for y in range(128):
            yg = 128 * t + y
            v0, q = yg // 4, yg % 4
            for d in range(5):
                v = v0 + d - 2
                if 0 <= v < 128:
                    bands[t, v, y] = Wp[q, d]
    return bands


def _poly(a):
    """[..., 288] -> [..., 4p, 72j] with x = 4j + p."""
    return np.ascontiguousarray(
        a.reshape(*a.shape[:-1], JL, 4).swapaxes(-1, -2))


def _host_inputs(frame1, frame2):
    """Build the 8 per-core input maps."""
    bands = _band_matrices()
    # PE shift matrices: out[i] = sum_p lhsT[p, i] * in[p]
    shdn = np.zeros((128, 128), np.float32)   # out[i] = in[i-1]
    shup = np.zeros((128, 128), np.float32)   # out[i] = in[i+1]
    for i in range(1, 128):
        shdn[i - 1, i] = 1.0
        shup[i, i - 1] = 1.0
    in_maps = []
    for b in range(B):
        for w in range(2):
            xbase = 256 * w - 16
            sl1 = np.zeros((3, H, XL), np.float32)
            sl2 = np.zeros((3, H, XL), np.float32)
            lo, hi = max(0, xbase), min(W, xbase + XL)
            sl1[:, :, lo - xbase:hi - xbase] = frame1[b][:, :, lo:hi]
            sl2[:, :, lo - xbase:hi - xbase] = frame2[b][:, :, lo:hi]
            # [3c, 512, 288] -> [128v, 4ry, 3c, 4p, 72j]
            il1 = np.ascontiguousarray(
                _poly(sl1).reshape(3, 128, 4, 4, JL).transpose(1, 2, 0, 3, 4))
            il2 = np.ascontiguousarray(
                _poly(sl2).reshape(3, 128, 4, 4, JL).transpose(1, 2, 0, 3, 4))
            # column-validity mask for the gray plane, polyphase, ry-tiled
            xcols = xbase + np.arange(XL)
            valid = (xcols >= 0) & (xcols < W)
            xm = _poly(np.where(valid, POS, NEG).astype(np.float32))  # [4,72]
            xmask = np.tile(xm.reshape(-1), 4)                        # ry x 4
            # anchor-validity mask
            gxg = 64 * w - 2 + np.arange(GXL)
            gm = ((gxg >= 0) & (gxg < 128)).astype(np.float32)
            ws = np.repeat((15.0 - np.arange(15, dtype=np.float32)),
                           GXL)
            wsy = np.repeat(15.0 - np.array(DYIDX, dtype=np.float32),
                            GXL)
            row = np.concatenate([xmask, gm, np.zeros(512, np.float32),
                                  ws, np.zeros(256, np.float32), wsy])
            consts = np.tile(row[None, :], (128, 1))
            consts[:, OFF_BANDS:OFF_BANDS + 512] = \
                bands.transpose(1, 0, 2).reshape(128, 512)
            consts[:, OFF_SHDN:OFF_SHDN + 128] = shdn
            consts[:, OFF_SHUP:OFF_SHUP + 128] = shup
            in_maps.append({"f1s": il1.reshape(128, -1),
                            "f2s": il2.reshape(128, -1),
                            "consts": consts})
    return in_maps


# ----------------------------------------------------------------------------
# device program
# ----------------------------------------------------------------------------

def build_program():
    nc = bacc.Bacc("TRN2", target_bir_lowering=False, debug=False)

    f1s_d = nc.dram_tensor("f1s", [128, 4 * 3 * 4 * JL], F32,
                           kind="ExternalInput")
    f2s_d = nc.dram_tensor("f2s", [128, 4 * 3 * 4 * JL], F32,
                           kind="ExternalInput")
    consts_d = nc.dram_tensor("consts", [128, NCONST], F32,
                              kind="ExternalInput")
    out_d = nc.dram_tensor("out", [128, 4, 2, 256], F32,
                           kind="ExternalOutput")

    with tile.TileContext(nc) as tc:
        with tc.tile_pool(name="main", bufs=1) as pool, \
             tc.tile_pool(name="psum", bufs=4, space="PSUM") as psum_pool:

            raw1 = pool.tile([128, 4, 3, 4, JL], F32)
            raw2 = pool.tile([128, 4, 3, 4, JL], F32)
            feat2 = pool.tile([128, 4, 3, 4, JL], F32)
            gray1 = pool.tile([128, 4, 4, JL], F32)
            sd = pool.tile([128, 4, 2, 4, JL], F32)     # s=0, d=1
            sd1 = pool.tile([128, 4, 2, JL], F32)       # f1, phase 0 only
            q_t = pool.tile([128, 4, 4, JL], F32)
            r0_t = pool.tile([128, 4, 4, JL], F32)
            a_t = pool.tile([128, 4, 4, JL], F32)
            consts = pool.tile([128, NCONST], F32)
            f2px = pool.tile([128, 11, 3, 4, JL], F32)
            f1a = pool.tile([128, 3, JL], F32)
            # correlation scratch; aliases of tensors dead by corr time
            s2_t = pool.tile([128, 16, GXL], F32, tag="a_t")
            prodG = pool.tile([128, 4, 4, 3, GXL], F32, tag="raw2")
            pm0 = pool.tile([128, 4, 11, GXL], F32)
            pm1 = pool.tile([128, 4, 11, GXL], F32)
            cEg = pool.tile([128, 4, 4, GXL], F32)
            mG = pool.tile([128, 4, 2, GXL], F32)
            m8 = pool.tile([128, 2, 8, GXL], F32)
            rowmax = pool.tile([128, 16, GXL], F32, tag="q_t")
            colmax = pool.tile([128, 15, GXL], F32, tag="r0_t")
            colmaxE = pool.tile([128, 15, GXL], F32)
            wsum = pool.tile([128, 15, GXL], F32)
            m_t = pool.tile([128, GXL], F32)
            fm_t = pool.tile([128, GXL], F32)
            grid = pool.tile([128, 2, GXL], F32)
            hp = pool.tile([128, 2, 256], F32)
            hsc = pool.tile([128, 2, 256], F32)
            tscr = pool.tile([128, 64], F32)
            bands2 = pool.tile([128, 4, 128], F32)
            # V-pass scratch aliases f2px (dead after main corr loop)
            vps = pool.tile([128, 4, 8, 256], F32, tag="f2px")
            outsb = pool.tile([128, 4, 2, 256], F32, tag="raw1")

            TT = nc.vector.tensor_tensor
            STT = nc.vector.scalar_tensor_tensor
            _touch_n = [0]

            def touch(ap):
                k = _touch_n[0] = _touch_n[0] + 1
                nc.vector.tensor_copy(tscr[:][32:33, k % 64:k % 64 + 1], ap)

            # ---------------- input DMAs ----------------
            nc.sync.dma_start(consts[:][:, OFF_SHDN:OFF_SHDN + 256],
                              consts_d.ap()[:, OFF_SHDN:OFF_SHDN + 256])
            # raw2 arrives as 4 ry-plane pieces so gray can start on the
            # first plane while the rest stream in (input BW ~160GB/s)
            f2s_r = f2s_d.ap().rearrange("p (r k) -> p r k", r=4)
            f2s_rc = f2s_d.ap().rearrange("p (r c k) -> p r c k", r=4, c=3)
            nc.sync.dma_start(raw2[:][:, 0, 0], f2s_rc[:, 0, 0])
            nc.sync.dma_start(raw2[:][:, 0, 1:3], f2s_rc[:, 0, 1:3])
            for ry in range(1, 4):
                nc.sync.dma_start(raw2[:][:, ry], f2s_r[:, ry])
            for p0 in range(0, 128, 32):
                nc.sync.dma_start(
                    raw1[:].rearrange("p r c q j -> p (r c q j)")[p0:p0 + 32],
                    f1s_d.ap()[p0:p0 + 32])
            # shift matrices first (tiny, unblocks the PE shift matmuls
            # ~20us before the bulk consts lands); the bulk pieces exclude
            # that region so no overlapping write re-orders the PE reads
            nc.sync.dma_start(consts[:][:, 0:OFF_SHDN],
                              consts_d.ap()[:, 0:OFF_SHDN])
            nc.sync.dma_start(consts[:][:, OFF_WSY:],
                              consts_d.ap()[:, OFF_WSY:])
            touch(consts[:][32:33, 0:1])
            # preload the abs_reciprocal_sqrt act table (covers Square and
            # Copy too) so no mid-pipeline ACT_TABLE_LOAD occurs
            nc.scalar.activation(tscr[:][0:1, 0:2], tscr[:][0:1, 2:4],
                                 Act.Abs_reciprocal_sqrt)
            xmask = consts[:][:, OFF_XMASK:OFF_XMASK + 1152].rearrange(
                "p (r q j) -> p r q j", r=4, q=4)
            gmask = consts[:][:, OFF_GMASK:OFF_GMASK + GXL]
            bands = consts[:][:, OFF_BANDS:OFF_BANDS + 512].rearrange(
                "p (t y) -> p t y", t=4)
            wslot = consts[:][:, OFF_WS:OFF_WS + 15 * GXL].rearrange(
                "p (s g) -> p s g", s=15)
            wsloty = consts[:][:, OFF_WSY:OFF_WSY + 15 * GXL].rearrange(
                "p (s g) -> p s g", s=15)
            shdn = consts[:][:, OFF_SHDN:OFF_SHDN + 128]
            shup = consts[:][:, OFF_SHUP:OFF_SHUP + 128]
            # stage the V-pass band matrices out of the critical path
            nc.vector.tensor_copy(bands2[:], bands)

            # f2px out-of-image corners (overwritten where valid by DMA);
            # gpsimd memsets overlap the input DMA wait
            nc.gpsimd.memset(f2px[:][0:2, 0:7, 0, :, :], float(NEG))
            nc.gpsimd.memset(f2px[:][0:2, 0:7, 1:3, :, :], 0.0)
            nc.gpsimd.memset(f2px[:][96:128, 7:11, 0, :, :], float(NEG))
            nc.gpsimd.memset(f2px[:][96:128, 7:11, 1:3, :, :], 0.0)

            # ---------------- frame2 features ----------------
            g2 = feat2[:][:, :, 0, :, :]                 # [128, 4ry, 4p, 72]
            r2v = raw2[:]
            # per-ry gray so each chunk starts when its input plane lands
            for ry in range(4):
                nc.vector.tensor_scalar_mul(g2[:, ry], r2v[:, ry, 0, :, :],
                                            0.299)
                STT(g2[:, ry], r2v[:, ry, 1, :, :], 0.587, g2[:, ry],
                    Alu.mult, Alu.add)
                STT(g2[:, ry], r2v[:, ry, 2, :, :], 0.114, g2[:, ry],
                    Alu.mult, Alu.add)
            # gray^2 on ACT, overlapped with the H-pass below
            nc.scalar.activation(q_t[:], g2, Act.Square)

            s_ = sd[:][:, :, 0, :, :]                    # [128, 4ry, 4p, 72]
            d_ = sd[:][:, :, 1, :, :]
            # d = g(x+1) - g(x-1); s = g(x-1) + 2 g(x) + g(x+1)
            TT(d_[:, :, 1:3, :], g2[:, :, 2:4, :], g2[:, :, 0:2, :],
               Alu.subtract)
            TT(d_[:, :, 0, 1:JL], g2[:, :, 1, 1:JL], g2[:, :, 3, 0:JL - 1],
               Alu.subtract)
            TT(d_[:, :, 3, 0:JL - 1], g2[:, :, 0, 1:JL], g2[:, :, 2, 0:JL - 1],
               Alu.subtract)
            STT(s_[:, :, 1:3, :], g2[:, :, 1:3, :], 2.0, g2[:, :, 0:2, :],
                Alu.mult, Alu.add)
            TT(s_[:, :, 1:3, :], s_[:, :, 1:3, :], g2[:, :, 2:4, :], Alu.add)
            STT(s_[:, :, 0, 1:JL], g2[:, :, 0, 1:JL], 2.0,
                g2[:, :, 3, 0:JL - 1], Alu.mult, Alu.add)
            TT(s_[:, :, 0, 1:JL], s_[:, :, 0, 1:JL], g2[:, :, 1, 1:JL],
               Alu.add)
            STT(s_[:, :, 3, 0:JL - 1], g2[:, :, 3, 0:JL - 1], 2.0,
                g2[:, :, 2, 0:JL - 1], Alu.mult, Alu.add)
            TT(s_[:, :, 3, 0:JL - 1], s_[:, :, 3, 0:JL - 1],
               g2[:, :, 0, 1:JL], Alu.add)
            # x boundary columns (x=0 and x=287)
            nc.vector.memset(sd[:][:, :, :, 0, 0:1], 0.0)
            nc.vector.memset(sd[:][:, :, :, 3, JL - 1:JL], 0.0)

            # cross-partition row shifts on PE: dn -> row 4v-1, up -> row 4v+4
            ps_sm1 = psum_pool.tile([128, 288], F32, tag="shift")
            ps_dm1 = psum_pool.tile([128, 288], F32, tag="shift")
            ps_sp1 = psum_pool.tile([128, 288], F32, tag="shift")
            ps_dp1 = psum_pool.tile([128, 288], F32, tag="shift")
            sd_f = sd[:].rearrange("p r s q j -> p r s (q j)")
            nc.tensor.matmul(ps_sm1[:], shdn, sd_f[:, 3, 0, :],
                             start=True, stop=True)
            nc.tensor.matmul(ps_dm1[:], shdn, sd_f[:, 3, 1, :],
                             start=True, stop=True)
            nc.tensor.matmul(ps_sp1[:], shup, sd_f[:, 0, 0, :],
                             start=True, stop=True)
            nc.tensor.matmul(ps_dp1[:], shup, sd_f[:, 0, 1, :],
                             start=True, stop=True)

            fx2 = feat2[:][:, :, 1, :, :]
            fy2 = feat2[:][:, :, 2, :, :]
            pq = lambda ps: ps[:].rearrange("p (q j) -> p q j", q=4)

            def vfx(ry, dm1, dp1):
                STT(fx2[:, ry, :, :], d_[:, ry, :, :], 2.0, dm1,
                    Alu.mult, Alu.add)
                TT(fx2[:, ry, :, :], fx2[:, ry, :, :], dp1, Alu.add)

            # fx planes first so the ACT square can start early
            vfx(1, d_[:, 0, :, :], d_[:, 2, :, :])
            vfx(2, d_[:, 1, :, :], d_[:, 3, :, :])
            vfx(0, pq(ps_dm1), d_[:, 1, :, :])
            vfx(3, d_[:, 2, :, :], pq(ps_dp1))
            HV = (slice(0, 2), slice(2, 4))
            for hf in HV:
                nc.scalar.activation(r0_t[:][:, hf], fx2[:, hf], Act.Square)
            TT(fy2[:, 1, :, :], s_[:, 2, :, :], s_[:, 0, :, :], Alu.subtract)
            TT(fy2[:, 0, :, :], s_[:, 1, :, :], pq(ps_sm1), Alu.subtract)
            nc.scalar.activation(a_t[:][:, 0:2], fy2[:, 0:2], Act.Square)
            TT(fy2[:, 2, :, :], s_[:, 3, :, :], s_[:, 1, :, :], Alu.subtract)
            TT(fy2[:, 3, :, :], pq(ps_sp1), s_[:, 2, :, :], Alu.subtract)
            nc.scalar.activation(a_t[:][:, 2:4], fy2[:, 2:4], Act.Square)

            # ---------------- frame2 normalize ----------------
            # processed in two ry-halves so ACT (square/rsqrt) overlaps the
            # DVE Newton/multiply chain of the other half
            for hf in HV:
                # q = max(fx2^2, 1e-24) + fy2^2 + gray^2
                STT(r0_t[:][:, hf], r0_t[:][:, hf], 1e-24, a_t[:][:, hf],
                    Alu.max, Alu.add)
                TT(q_t[:][:, hf], q_t[:][:, hf], r0_t[:][:, hf], Alu.add)
                nc.scalar.activation(r0_t[:][:, hf], q_t[:][:, hf],
                                     Act.Abs_reciprocal_sqrt)
            for hf in HV:
                # Newton: r1 = r0*(1.5 - 0.5*q*r0^2)
                TT(a_t[:][:, hf], r0_t[:][:, hf], r0_t[:][:, hf], Alu.mult)
                TT(a_t[:][:, hf], a_t[:][:, hf], q_t[:][:, hf], Alu.mult)
                nc.vector.tensor_scalar(a_t[:][:, hf], a_t[:][:, hf],
                                        -0.5, 1.5, Alu.mult, Alu.add)
                TT(r0_t[:][:, hf], r0_t[:][:, hf], a_t[:][:, hf], Alu.mult)
            for c in range(3):
                TT(feat2[:][:, :, c, :, :], feat2[:][:, :, c, :, :],
                   r0_t[:], Alu.mult)
            TT(g2, g2, xmask, Alu.min)


            # ---------------- f2px replication ----------------
            # ov=-2 (slots 1:4, slot 0 unused) and ov=-1 (slots 4:8) go over
            # DMA in 16-partition chunks spread across all queues; the ov=+1
            # group (slots 12:16) is built on the idle PE via shift-matmuls
            # with ACT copying PSUM->SBUF.
            f2px_f = f2px[:].rearrange("p s c q j -> p s (c q j)")
            feat2_f = feat2[:].rearrange("p r c q j -> p r (c q j)")
            for q0 in range(0, 128, 16):
                a, b = max(2, q0), q0 + 16
                if a < b:
                    nc.sync.dma_start(f2px_f[a:b, 0:3, :],
                                      feat2_f[a - 2:b - 2, 1:4, :])
            # ov=-1 (slots 3:7, shdn) and ov=+1 (slots 7:11, shup) via PE.
            # shdn copies all 128 partitions (row 0 gets the shift-matrix
            # zeros; gray is patched to NEG below); shup copies [0:127]
            # so partition 127 keeps its pre-set NEG/0 corner.
            for mat, dst0, pn in ((shdn, 3, 128), (shup, 7, 127)):
                for ry in range(4):
                    psa = psum_pool.tile([128, 512], F32, tag="vps")
                    psb = psum_pool.tile([128, 352], F32, tag="vps")
                    nc.tensor.matmul(psa[:], mat, feat2_f[:, ry, 0:512],
                                     start=True, stop=True)
                    nc.tensor.matmul(psb[:], mat, feat2_f[:, ry, 512:864],
                                     start=True, stop=True)
                    nc.scalar.copy(f2px_f[0:pn, dst0 + ry, 0:512],
                                   psa[:][0:pn])
                    nc.scalar.copy(f2px_f[0:pn, dst0 + ry, 512:864],
                                   psb[:][0:pn])
            nc.gpsimd.memset(f2px[:][0:1, 3:7, 0, :, :], float(NEG))

            # ---------------- frame1 features (anchors only) -------------
            g1 = gray1[:]
            r1v = raw1[:]
            # only ry planes {0,1,3} feed the anchor-row features
            for sl in (slice(0, 2), slice(3, 4)):
                nc.vector.tensor_scalar_mul(g1[:, sl],
                                            r1v[:, sl, 0, :, :], 0.299)
                STT(g1[:, sl], r1v[:, sl, 1, :, :], 0.587, g1[:, sl],
                    Alu.mult, Alu.add)
                STT(g1[:, sl], r1v[:, sl, 2, :, :], 0.114, g1[:, sl],
                    Alu.mult, Alu.add)
            s1 = sd1[:][:, :, 0, :]
            d1 = sd1[:][:, :, 1, :]
            # phase-0 H-pass only: d = g[p1,j] - g[p3,j-1]
            #                      s = g[p3,j-1] + 2 g[p0,j] + g[p1,j]
            TT(d1[:, :, 1:JL], g1[:, :, 1, 1:JL], g1[:, :, 3, 0:JL - 1],
               Alu.subtract)
            STT(s1[:, :, 1:JL], g1[:, :, 0, 1:JL], 2.0,
                g1[:, :, 3, 0:JL - 1], Alu.mult, Alu.add)
            TT(s1[:, :, 1:JL], s1[:, :, 1:JL], g1[:, :, 1, 1:JL], Alu.add)
            ps1 = psum_pool.tile([128, 2 * JL], F32, tag="shift")
            nc.tensor.matmul(ps1[:], shdn,
                             sd1[:][:, 3, :, :].rearrange("p s j -> p (s j)"),
                             start=True, stop=True)
            # f1a: c0 = gray, c1 = fx, c2 = fy  (anchor row ry=0, phase p=0)
            nc.vector.tensor_copy(f1a[:][:, 0, :], g1[:, 0, 0, :])
            STT(f1a[:][:, 1, 1:JL], d1[:, 0, 1:JL], 2.0,
                ps1[:][:, JL + 1:2 * JL], Alu.mult, Alu.add)
            TT(f1a[:][:, 1, 1:JL], f1a[:][:, 1, 1:JL], d1[:, 1, 1:JL],
               Alu.add)
            TT(f1a[:][:, 2, 1:JL], s1[:, 1, 1:JL], ps1[:][:, 1:JL],
               Alu.subtract)

            f1v = f1a[:][:, :, 2:2 + GXL]                   # [128, 3, 68]
            f1b4 = f1v.unsqueeze(1).broadcast_to([128, 4, 3, GXL])
            f1b7 = f1v.unsqueeze(1).broadcast_to([128, 7, 3, GXL])

            # ---------------- correlation: early phase (ov=0) -------------
            # slots 8-11 read feat2 directly, overlapping the f2px fill.
            # dx values sharing a window phase pw batch into one group:
            # their windows are overlapping stride-1 views (j0, j0+1, ...)
            s2g4 = s2_t[:].rearrange("p (d r) g -> p d r g", d=4)
            for gi, (pw, j0, ndx) in enumerate(
                    ((1, 0, 4), (2, 0, 4), (3, 0, 4), (0, 1, 3))):
                for c in range(3):
                    base = feat2[:][:, :, c, pw,
                                    j0:j0 + GXL].unsqueeze(1)
                    pr = list(base.ap)
                    win = AP(base.tensor, base.offset,
                             [pr[0], (1, ndx)] + pr[2:])
                    f1bc = f1a[:][:, c, 2:2 + GXL].unsqueeze(1).unsqueeze(
                        1).broadcast_to([128, ndx, 4, GXL])
                    TT(prodG[:][:, 0:ndx, :, c, :], f1bc, win, Alu.mult)
                TT(s2g4[:, 0:ndx], prodG[:][:, 0:ndx, :, 0, :],
                   prodG[:][:, 0:ndx, :, 1, :], Alu.add)
                TT(cEg[:][:, 0:ndx], s2g4[:, 0:ndx],
                   prodG[:][:, 0:ndx, :, 2, :], Alu.add)
                # rowmax: reduce over the dx group (overlap slice for n=3)
                TT(m8[:][:, :, 0:4, :], cEg[:][:, 0:2],
                   cEg[:][:, ndx - 2:ndx], Alu.max)
                TT(m8[:][:, 0, 4:8, :], m8[:][:, 0, 0:4, :],
                   m8[:][:, 1, 0:4, :], Alu.max)
                if gi == 0:
                    nc.vector.tensor_copy(rowmax[:][:, 11:15, :],
                                          m8[:][:, 0, 4:8, :])
                else:
                    TT(rowmax[:][:, 11:15, :], rowmax[:][:, 11:15, :],
                       m8[:][:, 0, 4:8, :], Alu.max)
                # colmaxE: reduce over ry per dx; slots gi::4 strided
                TT(mG[:][:, 0:ndx], cEg[:][:, 0:ndx, 0:2, :],
                   cEg[:][:, 0:ndx, 2:4, :], Alu.max)
                dx0 = (3 if pw == 0 else pw - 1)
                TT(colmaxE[:][:, dx0:dx0 + 4 * ndx - 3:4, :],
                   mG[:][:, 0:ndx, 0, :], mG[:][:, 0:ndx, 1, :], Alu.max)

            # ---------------- correlation: main phase ----------------
            # grouped by window phase pw like the early loop: per group the
            # three channel mults read overlapping stride-1 windows of all
            # 11 f2px slots, then adds/rowmax/colmax trees batch over the
            # whole dx group
            pm0v, pm1v = pm0[:], pm1[:]

            def mwin(c, pw, j0, ndx):
                base = f2px[:][:, :, c, pw, j0:j0 + GXL].unsqueeze(1)
                pr = list(base.ap)
                return AP(base.tensor, base.offset,
                          [pr[0], (1, ndx)] + pr[2:])

            for gi, (pw, j0, ndx) in enumerate(
                    ((1, 0, 4), (2, 0, 4), (3, 0, 4), (0, 1, 3))):
                dx0 = (3 if pw == 0 else pw - 1)
                for c, dst in ((0, pm0v), (1, pm1v)):
                    f1bc = f1a[:][:, c, 2:2 + GXL].unsqueeze(1).unsqueeze(
                        1).broadcast_to([128, ndx, 11, GXL])
                    TT(dst[:, 0:ndx], f1bc, mwin(c, pw, j0, ndx), Alu.mult)
                TT(pm0v[:, 0:ndx], pm0v[:, 0:ndx], pm1v[:, 0:ndx], Alu.add)
                f1bc = f1a[:][:, 2, 2:2 + GXL].unsqueeze(1).unsqueeze(
                    1).broadcast_to([128, ndx, 11, GXL])
                TT(pm1v[:, 0:ndx], f1bc, mwin(2, pw, j0, ndx), Alu.mult)
                TT(pm0v[:, 0:ndx], pm0v[:, 0:ndx], pm1v[:, 0:ndx], Alu.add)
                # rowmax: reduce over the dx group (pm1 rows are free now)
                TT(pm1v[:, 0:2], pm0v[:, 0:2], pm0v[:, ndx - 2:ndx],
                   Alu.max)
                TT(pm1v[:, 0], pm1v[:, 0], pm1v[:, 1], Alu.max)
                if gi == 0:
                    nc.vector.tensor_copy(rowmax[:][:, 0:11, :],
                                          pm1v[:, 0])
                else:
                    TT(rowmax[:][:, 0:11, :], rowmax[:][:, 0:11, :],
                       pm1v[:, 0], Alu.max)
                # colmax: reduce over the 11 slots per dx (overlap slices)
                TT(cEg[:][:, 0:ndx], pm0v[:, 0:ndx, 0:4, :],
                   pm0v[:, 0:ndx, 7:11, :], Alu.max)
                TT(cEg[:][:, 0:ndx], cEg[:][:, 0:ndx],
                   pm0v[:, 0:ndx, 3:7, :], Alu.max)
                TT(mG[:][:, 0:ndx], cEg[:][:, 0:ndx, 0:2, :],
                   cEg[:][:, 0:ndx, 2:4, :], Alu.max)
                TT(colmax[:][:, dx0:dx0 + 4 * ndx - 3:4, :],
                   mG[:][:, 0:ndx, 0, :], mG[:][:, 0:ndx, 1, :], Alu.max)

            # ---------------- argmax -> displacement grid ----------------
            # global max m_t is shared: max_dy rowmax == max_dx colmax
            TT(m8[:][:, 0], rowmax[:][:, 0:8, :], rowmax[:][:, 7:15, :],
               Alu.max)
            TT(m8[:][:, 0, 0:4, :], m8[:][:, 0, 0:4, :],
               m8[:][:, 0, 4:8, :], Alu.max)
            TT(m8[:][:, 0, 0:2, :], m8[:][:, 0, 0:2, :],
               m8[:][:, 0, 2:4, :], Alu.max)
            TT(m_t[:], m8[:][:, 0, 0, :], m8[:][:, 0, 1, :], Alu.max)
            mb = m_t[:].unsqueeze(1).broadcast_to([128, 15, GXL])

            def first_argmax(buf15, ch, wsl):
                # buf15: [128, 15, GXL] AP, slots = index 0..14; first
                # (smallest-index) argmax via is_ge * descending weights
                TT(wsum[:], buf15, mb, Alu.is_ge)
                TT(wsum[:], wsum[:], wsl, Alu.mult)
                TT(m8[:][:, 0], wsum[:][:, 0:8, :], wsum[:][:, 7:15, :],
                   Alu.max)
                TT(m8[:][:, 0, 0:4, :], m8[:][:, 0, 0:4, :],
                   m8[:][:, 0, 4:8, :], Alu.max)
                TT(m8[:][:, 0, 0:2, :], m8[:][:, 0, 0:2, :],
                   m8[:][:, 0, 2:4, :], Alu.max)
                TT(fm_t[:], m8[:][:, 0, 0, :], m8[:][:, 0, 1, :], Alu.max)
                # disp = (argmax-7)/512 = (8 - fm)/512 ; zero invalid anchors
                nc.vector.tensor_scalar(fm_t[:], fm_t[:], -1.0 / 512.0,
                                        8.0 / 512.0, Alu.mult, Alu.add)
                TT(grid[:][:, ch, :], fm_t[:], gmask, Alu.mult)

            first_argmax(rowmax[:][:, 0:15, :], 1, wsloty)
            TT(colmax[:], colmax[:], colmaxE[:], Alu.max)
            first_argmax(colmax[:], 0, wslot)

            # ---------------- smoothing H-pass (phase weights) -------------
            # 4 independent accumulation chains, emitted interleaved so
            # consecutive DVE instructions are independent
            Wp = _phase_weights()
            hscp = [hsc[:][:, :, 0:64], hsc[:][:, :, 64:128],
                    hsc[:][:, :, 128:192], hsc[:][:, :, 192:256]]
            for p in range(4):
                nc.vector.tensor_scalar_mul(
                    hscp[p], grid[:][:, :, 0:64], float(Wp[p, 0]))
            for dd in range(1, 4):
                for p in range(4):
                    STT(hscp[p], grid[:][:, :, dd:dd + 64],
                        float(Wp[p, dd]), hscp[p], Alu.mult, Alu.add)
            for p in range(4):
                STT(hp[:][:, :, p:256:4], grid[:][:, :, 4:4 + 64],
                    float(Wp[p, 4]), hscp[p], Alu.mult, Alu.add)

            # ---------------- V-pass (PE banded matmul) + normalize --------
            rhs = hp[:].rearrange("p c x -> p (c x)")
            for t in range(4):
                ps = psum_pool.tile([128, 512], F32, tag="vps")
                if t == 0:
                    nc.tensor.matmul(ps[:][:, 0:256], bands2[:][:, t, :],
                                     rhs[:, 0:256], start=True, stop=True)
                    nc.tensor.matmul(ps[:][:, 256:512], bands2[:][:, t, :],
                                     rhs[:, 256:512], start=True, stop=True)
                else:
                    nc.tensor.matmul(ps[:], bands2[:][:, t, :], rhs,
                                     start=True, stop=True)
                v = vps[:][:, t]
                sqx, sqy = v[:, 0, :], v[:, 1, :]
                nq, nm = v[:, 2, :], v[:, 3, :]
                nq2, nr2 = v[:, 4, :], v[:, 5, :]
                nc.scalar.activation(sqx, ps[:][:, 0:256], Act.Square)
                nc.scalar.activation(sqy, ps[:][:, 256:512], Act.Square)
                # q = max(qx,1e-30)+qy ; mag = q * rsqrt(q)
                STT(nq, sqx, 1e-30, sqy, Alu.max, Alu.add)
                nc.scalar.activation(nm, nq, Act.Abs_reciprocal_sqrt)
                TT(nm, nm, nq, Alu.mult)
                # magc = max(mag,1e-6)+1e-6 ; 1/magc = ars(magc^2)
                nc.vector.tensor_scalar(nm, nm, 1e-6, 1e-6, Alu.max, Alu.add)
                TT(nq2, nm, nm, Alu.mult)
                nc.scalar.activation(nr2, nq2, Act.Abs_reciprocal_sqrt)
                TT(outsb[:][:, t, 0, :], ps[:][:, 0:256], nr2, Alu.mult)
                TT(outsb[:][:, t, 1, :], ps[:][:, 256:512], nr2, Alu.mult)
                if t < 3:
                    nc.sync.dma_start(out_d.ap()[:, t:t + 1],
                                      outsb[:][:, t:t + 1])
                else:
                    # final block in two halves, issued from two sequencers
                    nc.sync.dma_start(out_d.ap()[0:64, t:t + 1],
                                      outsb[:][0:64, t:t + 1])
                    nc.scalar.dma_start(out_d.ap()[64:128, t:t + 1],
                                        outsb[:][64:128, t:t + 1])

    nc.compile()
    return nc


_NC_CACHE = None


def _get_nc():
    global _NC_CACHE
    if _NC_CACHE is None:
        _NC_CACHE = build_program()
    return _NC_CACHE


def kernel(frame1, frame2):
    frame1 = np.asarray(frame1, dtype=np.float32)
    frame2 = np.asarray(frame2, dtype=np.float32)
    nc = _get_nc()
    in_maps = _host_inputs(frame1, frame2)
    res = run_bass_kernel_spmd(nc, in_maps, core_ids=list(range(8)))
    if res.exec_time_ns is not None:
        print(f"HW exec time: {res.exec_time_ns} ns")
    out = np.empty((B, 2, H, W), np.float32)
    for b in range(B):
        for w in range(2):
            o = res.results[2 * b + w]["out"]        # [128, 4, 2, 256]
            o = o.transpose(2, 1, 0, 3).reshape(2, H, 256)
            out[b, :, :, 256 * w:256 * w + 256] = o
    return out
